# revision 7
# baseline (speedup 1.0000x reference)
"""Multi-head attention (B=8, N=1024, C=768, H=12) for 8 Trainium2 NeuronCores.

Sharding: data-parallel over the batch dim — core b computes batch element b.
Weights are replicated; no collectives.

Per-core plan (all layouts picked so that NO on-device transposes are needed):
  host feeds xT=[C,N] (x[b].T), wqkvT=[C,3C], wprojT=[C,C], bproj=[C].
  1. qT/kT GEMM:  qkT[d, n] = wqkvT_chunk.T @ xT          (d on partitions)
  2. V GEMM:      V[n, dv]  = xT_chunk.T @ wqkvT[:, 2C:]  (natural layout),
                  stored head-strided as V''[n, 12*(64+1)] with a ones column
                  per head (the ones row of V''.T yields the softmax
                  denominator for free during the PV matmul).
  3. Attention per head pair (2 heads packed into the 128-wide PE array by
     row tiling, since head_dim=64):
       S^T[nk, nq]  = kT_chunk.T @ qT          (one matmul per nk chunk)
       expS         = exp(SCALE * S^T)         (ACT engine, PSUM -> SBUF;
                                               max-free softmax: scaled
                                               scores are ~N(0,1), fp32-safe)
       O'[65, nq]  += V''_chunk.T @ expS       (rows 0:64 = unnormalized out^T,
                                               row 64 = sum of exp = denom)
       attnT[c, n]  = O'[0:64] * (1/denom)     (reciprocal + partition-bcast)
  4. proj:        y[n, d2] = attnT_chunk.T @ wprojT + bproj
All matmuls run as float32r (full PE rate at N>=256, ~fp32 precision).
"""

import sys

for _p in ("/opt/trn_rl_repo", "/opt/pypackages"):
    if _p not in sys.path:
        sys.path.append(_p)

import numpy as np

import concourse.bass as bass
import concourse.tile as tile
from concourse import bacc, mybir
from concourse.bass_utils import run_bass_kernel_spmd

B, N, C = 8, 1024, 768
H, HD = 12, 64
SCALE = HD**-0.5
NCORES = 8
KC = C // 128  # 6 contraction chunks over C
NT = N // 128  # 8 chunks over sequence (nk / n-tiles)
NQT = N // 512  # 2 moving-dim tiles over the query sequence
PAIRS = H // 2  # 6 head pairs
F32 = mybir.dt.float32
F32R = mybir.dt.float32r
EXP = mybir.ActivationFunctionType.Exp


def _emit(tc, nc, xT, wqkvT, wprojT, bproj, y, ctx):
    persist = ctx.enter_context(tc.tile_pool(name="persist", bufs=1))
    wqk_pool = ctx.enter_context(tc.tile_pool(name="wqk", bufs=12))
    work = ctx.enter_context(tc.tile_pool(name="work", bufs=3))
    expp = ctx.enter_context(tc.tile_pool(name="expp", bufs=4))
    rdp = ctx.enter_context(tc.tile_pool(name="rdp", bufs=4))
    dram_scr = ctx.enter_context(tc.tile_pool(name="dram_scr", bufs=4, space="DRAM"))
    ps_main = ctx.enter_context(tc.tile_pool(name="ps_main", bufs=4, space="PSUM"))
    ps_s = ctx.enter_context(tc.tile_pool(name="ps_s", bufs=2, space="PSUM"))
    ps_o = ctx.enter_context(tc.tile_pool(name="ps_o", bufs=2, space="PSUM"))

    # ---- persistent loads ----
    xTs = []
    for kc in range(KC):
        t = persist.tile([128, N], F32R, tag=f"xT{kc}")
        nc.sync.dma_start(out=t, in_=xT[kc * 128 : (kc + 1) * 128, :])
        xTs.append(t)
    wvs = []
    for kc in range(KC):
        t = persist.tile([128, C], F32R, tag=f"wv{kc}")
        nc.sync.dma_start(out=t, in_=wqkvT[kc * 128 : (kc + 1) * 128, 2 * C : 3 * C])
        wvs.append(t)
    wps = []
    for kc in range(KC):
        t = persist.tile([128, C], F32R, tag=f"wp{kc}")
        nc.sync.dma_start(out=t, in_=wprojT[kc * 128 : (kc + 1) * 128, :])
        wps.append(t)
    bpb = persist.tile([128, C], F32, tag="bpb")
    nc.gpsimd.dma_start(
        out=bpb,
        in_=bass.AP(tensor=bproj.tensor, offset=bproj.offset, ap=[[0, 128]] + list(bproj.ap)),
    )

    # ---- phase B: V GEMM (natural layout, head-strided with ones column) ----
    v2s = []
    for nt in range(NT):
        v2s.append(persist.tile([128, H * (HD + 1)], F32R, tag=f"v2{nt}", name=f"v2{nt}"))
    for nt in range(NT):
        ps0 = ps_main.tile([128, 512], F32, tag="ps_main")
        ps1 = ps_main.tile([128, 256], F32, tag="ps_main")
        for kc in range(KC):
            lhsT = (xTs[kc][:, nt * 128 : (nt + 1) * 128])
            nc.tensor.matmul(
                ps0, lhsT, (wvs[kc][:, 0:512]), start=(kc == 0), stop=(kc == KC - 1)
            )
            nc.tensor.matmul(
                ps1, lhsT, (wvs[kc][:, 512:768]), start=(kc == 0), stop=(kc == KC - 1)
            )
        v2v = v2s[nt].rearrange("p (h e) -> p h e", e=HD + 1)
        nc.vector.tensor_copy(out=v2v[:, 0:8, 0:HD], in_=ps0.rearrange("p (h e) -> p h e", e=HD))
        nc.vector.tensor_copy(out=v2v[:, 8:12, 0:HD], in_=ps1.rearrange("p (h e) -> p h e", e=HD))
        nc.vector.memset(v2v[:, :, HD : HD + 1].bitcast(F32), 1.0)

    # ---- phases C (qk GEMM) + D (attention), interleaved per head pair ----
    aTs = []
    for j in range(PAIRS):
        qT = persist.tile([128, N], F32R, tag=f"qT{j}")
        kT = persist.tile([128, N], F32R, tag=f"kT{j}")
        for dst, dcol0 in ((qT, j * 128), (kT, C + j * 128)):
            ps = [ps_main.tile([128, 512], F32, tag="ps_main", name=f"psqk{h2}") for h2 in range(NQT)]
            for kc in range(KC):
                w = wqk_pool.tile([128, 128], F32R, tag="wqk")
                nc.sync.dma_start(
                    out=w, in_=wqkvT[kc * 128 : (kc + 1) * 128, dcol0 : dcol0 + 128]
                )
                for h2 in range(NQT):
                    nc.tensor.matmul(
                        ps[h2],
                        (w),
                        (xTs[kc][:, h2 * 512 : (h2 + 1) * 512]),
                        start=(kc == 0),
                        stop=(kc == KC - 1),
                    )
            for h2 in range(NQT):
                nc.vector.tensor_copy(out=dst[:, h2 * 512 : (h2 + 1) * 512], in_=ps[h2])

        aT = persist.tile([128, N], F32R, tag=f"aT{j}")
        aTs.append(aT)
        for nq in range(NQT):
            oA = ps_o.tile([HD + 1, 512], F32, tag="ps_o")
            oB = ps_o.tile([HD + 1, 512], F32, tag="ps_o")
            for nkc in range(NT):
                sA = ps_s.tile([128, 512], F32, tag="ps_s")
                sB = ps_s.tile([128, 512], F32, tag="ps_s")
                nc.tensor.matmul(
                    sA,
                    (kT[0:64, nkc * 128 : (nkc + 1) * 128]),
                    (qT[0:64, nq * 512 : (nq + 1) * 512]),
                    tile_position=(0, 0),
                )
                nc.tensor.matmul(
                    sB,
                    (kT[64:128, nkc * 128 : (nkc + 1) * 128]),
                    (qT[64:128, nq * 512 : (nq + 1) * 512]),
                    tile_position=(64, 0),
                )
                eA = expp.tile([128, 512], F32R, tag="expp")
                eB = expp.tile([128, 512], F32R, tag="expp")
                nc.scalar.activation(out=eA, in_=sA, func=EXP, scale=SCALE)
                nc.scalar.activation(out=eB, in_=sB, func=EXP, scale=SCALE)
                v2v = v2s[nkc].rearrange("p (h e) -> p h e", e=HD + 1)
                nc.tensor.matmul(
                    oA, (v2v[:, 2 * j, :]), (eA), start=(nkc == 0), stop=(nkc == NT - 1)
                )
                nc.tensor.matmul(
                    oB, (v2v[:, 2 * j + 1, :]), (eB), start=(nkc == 0), stop=(nkc == NT - 1)
                )
            for o, half in ((oA, 0), (oB, 1)):
                rd = rdp.tile([1, 512], F32, tag="rd")
                nc.vector.reciprocal(out=rd, in_=o[HD : HD + 1, :])
                # SBUF APs cannot broadcast across partitions; bounce the
                # reciprocal row through DRAM and broadcast-read it back.
                scr = dram_scr.tile([1, 512], F32, tag="scr")
                nc.sync.dma_start(out=scr, in_=rd)
                rb = rdp.tile([64, 512], F32, tag="rb")
                nc.gpsimd.dma_start(
                    out=rb,
                    in_=bass.AP(tensor=scr.tensor, offset=scr.offset, ap=[[0, 64]] + list(scr.ap[1:])),
                )
                nc.vector.tensor_mul(
                    out=aT[half * 64 : half * 64 + 64, nq * 512 : (nq + 1) * 512],
                    in0=o[0:HD, :],
                    in1=rb,
                )

    # ---- phase E: proj + bias ----
    for nt in range(NT):
        ps0 = ps_main.tile([128, 512], F32, tag="ps_main")
        ps1 = ps_main.tile([128, 256], F32, tag="ps_main")
        for kc in range(KC):
            lhsT = (aTs[kc][:, nt * 128 : (nt + 1) * 128])
            nc.tensor.matmul(
                ps0, lhsT, (wps[kc][:, 0:512]), start=(kc == 0), stop=(kc == KC - 1)
            )
            nc.tensor.matmul(
                ps1, lhsT, (wps[kc][:, 512:768]), start=(kc == 0), stop=(kc == KC - 1)
            )
        yb = work.tile([128, C], F32, tag="yb")
        nc.vector.tensor_add(out=yb[:, 0:512], in0=ps0, in1=bpb[:, 0:512])
        nc.vector.tensor_add(out=yb[:, 512:768], in0=ps1, in1=bpb[:, 512:768])
        nc.sync.dma_start(out=y[nt * 128 : (nt + 1) * 128, :], in_=yb)


def build():
    from contextlib import ExitStack

    nc = bacc.Bacc("TRN2", target_bir_lowering=False, debug=False)
    xT = nc.dram_tensor("xT", [C, N], F32R, kind="ExternalInput").ap()
    wqkvT = nc.dram_tensor("wqkvT", [C, 3 * C], F32R, kind="ExternalInput").ap()
    wprojT = nc.dram_tensor("wprojT", [C, C], F32R, kind="ExternalInput").ap()
    bproj = nc.dram_tensor("bproj", [C], F32, kind="ExternalInput").ap()
    y = nc.dram_tensor("y", [N, C], F32, kind="ExternalOutput").ap()
    with tile.TileContext(nc) as tc:
        with ExitStack() as ctx:
            _emit(tc, nc, xT, wqkvT, wprojT, bproj, y, ctx)
    nc.compile()
    return nc


_NC_CACHE = {}


def make_in_maps(x, w_qkv, w_proj, b_proj):
    wqkvT = np.ascontiguousarray(np.asarray(w_qkv).T)
    wprojT = np.ascontiguousarray(np.asarray(w_proj).T)
    b_proj = np.asarray(b_proj)
    return [
        {
            "xT": np.ascontiguousarray(np.asarray(x[b]).T),
            "wqkvT": wqkvT,
            "wprojT": wprojT,
            "bproj": b_proj,
        }
        for b in range(NCORES)
    ]


def kernel(x, w_qkv, w_proj, b_proj, _trace=False, _tmpdir=None):
    if "nc" not in _NC_CACHE:
        _NC_CACHE["nc"] = build()
    nc = _NC_CACHE["nc"]
    in_maps = make_in_maps(x, w_qkv, w_proj, b_proj)
    kwargs = {}
    if _trace:
        kwargs = {"trace": True, "tmpdir": _tmpdir}
    res = run_bass_kernel_spmd(nc, in_maps, core_ids=list(range(NCORES)), **kwargs)
    out = np.stack([res.results[i]["y"] for i in range(NCORES)], axis=0)
    if _trace:
        _NC_CACHE["last_result"] = res
    return out


if __name__ == "__main__":
    rng = np.random.default_rng(0)
    x = rng.standard_normal((B, N, C), dtype=np.float32)
    w_qkv = (rng.standard_normal((3 * C, C), dtype=np.float32) * C**-0.5).astype(np.float32)
    w_proj = (rng.standard_normal((C, C), dtype=np.float32) * C**-0.5).astype(np.float32)
    b_proj = np.zeros(C, dtype=np.float32)
    out = kernel(x, w_qkv, w_proj, b_proj)
    print("out", out.shape, out.dtype, float(np.abs(out).mean()))


# revision 8
# speedup vs baseline: 1.0646x; 1.0646x over previous
"""Multi-head attention (B=8, N=1024, C=768, H=12) for 8 Trainium2 NeuronCores.

Sharding: data-parallel over the batch dim — core b computes batch element b.
Weights are replicated; no collectives.

Per-core plan (all layouts picked so that NO on-device transposes are needed):
  host feeds xT=[C,N] (x[b].T), wqkvT=[C,3C], wprojT=[C,C], bproj=[C].
  1. qT/kT GEMM:  qkT[d, n] = wqkvT_chunk.T @ xT          (d on partitions)
  2. V GEMM:      V[n, dv]  = xT_chunk.T @ wqkvT[:, 2C:]  (natural layout),
                  stored head-strided as V''[n, 12*(64+1)] with a ones column
                  per head (the ones row of V''.T yields the softmax
                  denominator for free during the PV matmul).
  3. Attention per head pair (2 heads packed into the 128-wide PE array by
     row tiling, since head_dim=64):
       S^T[nk, nq]  = kT_chunk.T @ qT          (one matmul per nk chunk)
       expS         = exp(SCALE * S^T)         (ACT engine, PSUM -> SBUF;
                                               max-free softmax: scaled
                                               scores are ~N(0,1), fp32-safe)
       O'[65, nq]  += V''_chunk.T @ expS       (rows 0:64 = unnormalized out^T,
                                               row 64 = sum of exp = denom)
       attnT[c, n]  = O'[0:64] * (1/denom)     (reciprocal + partition-bcast)
  4. proj:        y[n, d2] = attnT_chunk.T @ wprojT + bproj
All matmuls run as float32r (full PE rate at N>=256, ~fp32 precision).
"""

import sys

for _p in ("/opt/trn_rl_repo", "/opt/pypackages"):
    if _p not in sys.path:
        sys.path.append(_p)

import numpy as np

import concourse.bass as bass
import concourse.tile as tile
from concourse import bacc, mybir
from concourse.bass_utils import run_bass_kernel_spmd

B, N, C = 8, 1024, 768
H, HD = 12, 64
SCALE = HD**-0.5
NCORES = 8
KC = C // 128  # 6 contraction chunks over C
NT = N // 128  # 8 chunks over sequence (nk / n-tiles)
NQT = N // 512  # 2 moving-dim tiles over the query sequence
PAIRS = H // 2  # 6 head pairs
F32 = mybir.dt.float32
F32R = mybir.dt.float32r
BF16 = mybir.dt.bfloat16
EXP = mybir.ActivationFunctionType.Exp


def _emit(tc, nc, xT, wqkvT, wprojT, bproj, y, ctx):
    persist = ctx.enter_context(tc.tile_pool(name="persist", bufs=1))
    wqk_pool = ctx.enter_context(tc.tile_pool(name="wqk", bufs=12))
    work = ctx.enter_context(tc.tile_pool(name="work", bufs=3))
    expp = ctx.enter_context(tc.tile_pool(name="expp", bufs=4))
    rdp = ctx.enter_context(tc.tile_pool(name="rdp", bufs=4))
    dram_scr = ctx.enter_context(tc.tile_pool(name="dram_scr", bufs=4, space="DRAM"))
    ps_main = ctx.enter_context(tc.tile_pool(name="ps_main", bufs=4, space="PSUM"))
    ps_s = ctx.enter_context(tc.tile_pool(name="ps_s", bufs=2, space="PSUM"))
    ps_o = ctx.enter_context(tc.tile_pool(name="ps_o", bufs=2, space="PSUM"))

    # ---- persistent loads ----
    xTs = []
    for kc in range(KC):
        t = persist.tile([128, N], F32R, tag=f"xT{kc}")
        nc.sync.dma_start(out=t, in_=xT[kc * 128 : (kc + 1) * 128, :])
        xTs.append(t)
    wvs = []
    for kc in range(KC):
        t = persist.tile([128, C], F32R, tag=f"wv{kc}")
        nc.sync.dma_start(out=t, in_=wqkvT[kc * 128 : (kc + 1) * 128, 2 * C : 3 * C])
        wvs.append(t)
    wps = []
    for kc in range(KC):
        t = persist.tile([128, C], F32R, tag=f"wp{kc}")
        nc.sync.dma_start(out=t, in_=wprojT[kc * 128 : (kc + 1) * 128, :])
        wps.append(t)
    bpb = persist.tile([128, C], F32, tag="bpb")
    nc.gpsimd.dma_start(
        out=bpb,
        in_=bass.AP(tensor=bproj.tensor, offset=bproj.offset, ap=[[0, 128]] + list(bproj.ap)),
    )

    # ---- phase B: V GEMM (natural layout, head-strided with ones column) ----
    v2s = []
    for nt in range(NT):
        v2s.append(persist.tile([128, H * (HD + 1)], BF16, tag=f"v2{nt}", name=f"v2{nt}"))
    for nt in range(NT):
        ps0 = ps_main.tile([128, 512], F32, tag="ps_main")
        ps1 = ps_main.tile([128, 256], F32, tag="ps_main")
        for kc in range(KC):
            lhsT = (xTs[kc][:, nt * 128 : (nt + 1) * 128])
            nc.tensor.matmul(
                ps0, lhsT, (wvs[kc][:, 0:512]), start=(kc == 0), stop=(kc == KC - 1)
            )
            nc.tensor.matmul(
                ps1, lhsT, (wvs[kc][:, 512:768]), start=(kc == 0), stop=(kc == KC - 1)
            )
        v2v = v2s[nt].rearrange("p (h e) -> p h e", e=HD + 1)
        nc.vector.tensor_copy(out=v2v[:, 0:8, 0:HD], in_=ps0.rearrange("p (h e) -> p h e", e=HD))
        nc.vector.tensor_copy(out=v2v[:, 8:12, 0:HD], in_=ps1.rearrange("p (h e) -> p h e", e=HD))
        nc.vector.memset(v2v[:, :, HD : HD + 1], 1.0)

    # ---- phases C (qk GEMM) + D (attention), interleaved per head pair ----
    aTs = []
    for j in range(PAIRS):
        qT = persist.tile([128, N], BF16, tag=f"qT{j}")
        kT = persist.tile([128, N], BF16, tag=f"kT{j}")
        for dst, dcol0 in ((qT, j * 128), (kT, C + j * 128)):
            ps = [ps_main.tile([128, 512], F32, tag="ps_main", name=f"psqk{h2}") for h2 in range(NQT)]
            for kc in range(KC):
                w = wqk_pool.tile([128, 128], F32R, tag="wqk")
                nc.sync.dma_start(
                    out=w, in_=wqkvT[kc * 128 : (kc + 1) * 128, dcol0 : dcol0 + 128]
                )
                for h2 in range(NQT):
                    nc.tensor.matmul(
                        ps[h2],
                        (w),
                        (xTs[kc][:, h2 * 512 : (h2 + 1) * 512]),
                        start=(kc == 0),
                        stop=(kc == KC - 1),
                    )
            for h2 in range(NQT):
                nc.vector.tensor_copy(out=dst[:, h2 * 512 : (h2 + 1) * 512], in_=ps[h2])

        aT = persist.tile([128, N], F32R, tag=f"aT{j}")
        aTs.append(aT)
        for nq in range(NQT):
            oA = ps_o.tile([HD + 1, 512], F32, tag="ps_o")
            oB = ps_o.tile([HD + 1, 512], F32, tag="ps_o")
            for nkc in range(NT):
                sA = ps_s.tile([128, 512], F32, tag="ps_s")
                sB = ps_s.tile([128, 512], F32, tag="ps_s")
                nc.tensor.matmul(
                    sA,
                    (kT[0:64, nkc * 128 : (nkc + 1) * 128]),
                    (qT[0:64, nq * 512 : (nq + 1) * 512]),
                    tile_position=(0, 0),
                )
                nc.tensor.matmul(
                    sB,
                    (kT[64:128, nkc * 128 : (nkc + 1) * 128]),
                    (qT[64:128, nq * 512 : (nq + 1) * 512]),
                    tile_position=(64, 0),
                )
                eA = expp.tile([128, 512], BF16, tag="expp")
                eB = expp.tile([128, 512], BF16, tag="expp")
                nc.scalar.activation(out=eA, in_=sA, func=EXP, scale=SCALE)
                nc.scalar.activation(out=eB, in_=sB, func=EXP, scale=SCALE)
                v2v = v2s[nkc].rearrange("p (h e) -> p h e", e=HD + 1)
                nc.tensor.matmul(
                    oA, (v2v[:, 2 * j, :]), (eA), start=(nkc == 0), stop=(nkc == NT - 1)
                )
                nc.tensor.matmul(
                    oB, (v2v[:, 2 * j + 1, :]), (eB), start=(nkc == 0), stop=(nkc == NT - 1)
                )
            for o, half in ((oA, 0), (oB, 1)):
                # denom row -> SBUF, bounce through DRAM to spread the 512
                # values across 128 partitions (reciprocal is ~6 cyc/elem on a
                # single DVE lane; [128,4] uses 128 lanes), bounce back and
                # broadcast-read over 64 partitions.
                rd = rdp.tile([1, 512], F32, tag="rd")
                nc.vector.tensor_copy(out=rd, in_=o[HD : HD + 1, :])
                scr = dram_scr.tile([1, 512], F32, tag="scr")
                nc.sync.dma_start(out=scr, in_=rd)
                rs = rdp.tile([128, 4], F32, tag="rs")
                nc.sync.dma_start(
                    out=rs,
                    in_=bass.AP(tensor=scr.tensor, offset=scr.offset, ap=[[4, 128], [1, 4]]),
                )
                rs2 = rdp.tile([128, 4], F32, tag="rs2")
                nc.vector.reciprocal(out=rs2, in_=rs)
                scr2 = dram_scr.tile([1, 512], F32, tag="scr2")
                nc.sync.dma_start(
                    out=bass.AP(tensor=scr2.tensor, offset=scr2.offset, ap=[[4, 128], [1, 4]]),
                    in_=rs2,
                )
                rb = rdp.tile([64, 512], F32, tag="rb")
                nc.gpsimd.dma_start(
                    out=rb,
                    in_=bass.AP(tensor=scr2.tensor, offset=scr2.offset, ap=[[0, 64]] + list(scr2.ap[1:])),
                )
                nc.vector.tensor_mul(
                    out=aT[half * 64 : half * 64 + 64, nq * 512 : (nq + 1) * 512],
                    in0=o[0:HD, :],
                    in1=rb,
                )

    # ---- phase E: proj + bias ----
    for nt in range(NT):
        ps0 = ps_main.tile([128, 512], F32, tag="ps_main")
        ps1 = ps_main.tile([128, 256], F32, tag="ps_main")
        for kc in range(KC):
            lhsT = (aTs[kc][:, nt * 128 : (nt + 1) * 128])
            nc.tensor.matmul(
                ps0, lhsT, (wps[kc][:, 0:512]), start=(kc == 0), stop=(kc == KC - 1)
            )
            nc.tensor.matmul(
                ps1, lhsT, (wps[kc][:, 512:768]), start=(kc == 0), stop=(kc == KC - 1)
            )
        yb = work.tile([128, C], F32, tag="yb")
        nc.vector.tensor_add(out=yb[:, 0:512], in0=ps0, in1=bpb[:, 0:512])
        nc.vector.tensor_add(out=yb[:, 512:768], in0=ps1, in1=bpb[:, 512:768])
        nc.sync.dma_start(out=y[nt * 128 : (nt + 1) * 128, :], in_=yb)


def build():
    from contextlib import ExitStack

    nc = bacc.Bacc("TRN2", target_bir_lowering=False, debug=False)
    xT = nc.dram_tensor("xT", [C, N], F32R, kind="ExternalInput").ap()
    wqkvT = nc.dram_tensor("wqkvT", [C, 3 * C], F32R, kind="ExternalInput").ap()
    wprojT = nc.dram_tensor("wprojT", [C, C], F32R, kind="ExternalInput").ap()
    bproj = nc.dram_tensor("bproj", [C], F32, kind="ExternalInput").ap()
    y = nc.dram_tensor("y", [N, C], F32, kind="ExternalOutput").ap()
    with tile.TileContext(nc) as tc:
        with ExitStack() as ctx:
            _emit(tc, nc, xT, wqkvT, wprojT, bproj, y, ctx)
    nc.compile()
    return nc


_NC_CACHE = {}


def make_in_maps(x, w_qkv, w_proj, b_proj):
    wqkvT = np.ascontiguousarray(np.asarray(w_qkv).T)
    wprojT = np.ascontiguousarray(np.asarray(w_proj).T)
    b_proj = np.asarray(b_proj)
    return [
        {
            "xT": np.ascontiguousarray(np.asarray(x[b]).T),
            "wqkvT": wqkvT,
            "wprojT": wprojT,
            "bproj": b_proj,
        }
        for b in range(NCORES)
    ]


def kernel(x, w_qkv, w_proj, b_proj, _trace=False, _tmpdir=None):
    if "nc" not in _NC_CACHE:
        _NC_CACHE["nc"] = build()
    nc = _NC_CACHE["nc"]
    in_maps = make_in_maps(x, w_qkv, w_proj, b_proj)
    kwargs = {}
    if _trace:
        kwargs = {"trace": True, "tmpdir": _tmpdir}
    res = run_bass_kernel_spmd(nc, in_maps, core_ids=list(range(NCORES)), **kwargs)
    out = np.stack([res.results[i]["y"] for i in range(NCORES)], axis=0)
    if _trace:
        _NC_CACHE["last_result"] = res
    return out


if __name__ == "__main__":
    rng = np.random.default_rng(0)
    x = rng.standard_normal((B, N, C), dtype=np.float32)
    w_qkv = (rng.standard_normal((3 * C, C), dtype=np.float32) * C**-0.5).astype(np.float32)
    w_proj = (rng.standard_normal((C, C), dtype=np.float32) * C**-0.5).astype(np.float32)
    b_proj = np.zeros(C, dtype=np.float32)
    out = kernel(x, w_qkv, w_proj, b_proj)
    print("out", out.shape, out.dtype, float(np.abs(out).mean()))


# revision 9
# speedup vs baseline: 1.3232x; 1.2429x over previous
"""Multi-head attention (B=8, N=1024, C=768, H=12) for 8 Trainium2 NeuronCores.

Sharding: data-parallel over the batch dim — core b computes batch element b.
Weights are replicated; no collectives.

Per-core plan (all layouts picked so that NO on-device transposes are needed):
  host feeds xT=[C,N] (x[b].T), wqkvT=[C,3C], wprojT=[C,C], bproj=[C].
  1. qT/kT GEMM:  qkT[d, n] = wqkvT_chunk.T @ xT          (d on partitions)
  2. V GEMM:      V[n, dv]  = xT_chunk.T @ wqkvT[:, 2C:]  (natural layout),
                  stored head-strided as V''[n, 12*(64+1)] with a ones column
                  per head (the ones row of V''.T yields the softmax
                  denominator for free during the PV matmul).
  3. Attention per head pair (2 heads packed into the 128-wide PE array by
     row tiling, since head_dim=64):
       S^T[nk, nq]  = kT_chunk.T @ qT          (one matmul per nk chunk)
       expS         = exp(SCALE * S^T)         (ACT engine, PSUM -> SBUF;
                                               max-free softmax: scaled
                                               scores are ~N(0,1), fp32-safe)
       O'[65, nq]  += V''_chunk.T @ expS       (rows 0:64 = unnormalized out^T,
                                               row 64 = sum of exp = denom)
       attnT[c, n]  = O'[0:64] * (1/denom)     (reciprocal + partition-bcast)
  4. proj:        y[n, d2] = attnT_chunk.T @ wprojT + bproj
All matmuls run as float32r (full PE rate at N>=256, ~fp32 precision).
"""

import sys

for _p in ("/opt/trn_rl_repo", "/opt/pypackages"):
    if _p not in sys.path:
        sys.path.append(_p)

import numpy as np

import concourse.bass as bass
import concourse.tile as tile
from concourse import bacc, mybir
from concourse.bass_utils import run_bass_kernel_spmd

B, N, C = 8, 1024, 768
H, HD = 12, 64
SCALE = HD**-0.5
NCORES = 8
KC = C // 128  # 6 contraction chunks over C
NT = N // 128  # 8 chunks over sequence (nk / n-tiles)
NQT = N // 512  # 2 moving-dim tiles over the query sequence
PAIRS = H // 2  # 6 head pairs
F32 = mybir.dt.float32
F32R = mybir.dt.float32r
BF16 = mybir.dt.bfloat16
EXP = mybir.ActivationFunctionType.Exp


def _emit(tc, nc, xT, wqkvT, wprojT, bproj, y, ctx):
    persist = ctx.enter_context(tc.tile_pool(name="persist", bufs=1))
    wqk_pool = ctx.enter_context(tc.tile_pool(name="wqk", bufs=12))
    work = ctx.enter_context(tc.tile_pool(name="work", bufs=3))
    expp = ctx.enter_context(tc.tile_pool(name="expp", bufs=4))
    rdp = ctx.enter_context(tc.tile_pool(name="rdp", bufs=4))
    dram_scr = ctx.enter_context(tc.tile_pool(name="dram_scr", bufs=4, space="DRAM"))
    # 8 PSUM banks total: ps_big = 3 slots x [128,1024] (2 banks each),
    # ps_o = 2 slots x [65,512] (1 bank each).
    ps_big = ctx.enter_context(tc.tile_pool(name="ps_big", bufs=3, space="PSUM"))
    ps_o = ctx.enter_context(tc.tile_pool(name="ps_o", bufs=2, space="PSUM"))

    # ---- persistent loads ----
    xTs = []
    for kc in range(KC):
        t = persist.tile([128, N], F32R, tag=f"xT{kc}")
        nc.sync.dma_start(out=t, in_=xT[kc * 128 : (kc + 1) * 128, :])
        xTs.append(t)
    wvs = []
    for kc in range(KC):
        t = persist.tile([128, C], F32R, tag=f"wv{kc}")
        nc.sync.dma_start(out=t, in_=wqkvT[kc * 128 : (kc + 1) * 128, 2 * C : 3 * C])
        wvs.append(t)
    wps = []
    for kc in range(KC):
        t = persist.tile([128, C], F32R, tag=f"wp{kc}")
        nc.sync.dma_start(out=t, in_=wprojT[kc * 128 : (kc + 1) * 128, :])
        wps.append(t)
    bpb = persist.tile([128, C], F32, tag="bpb")
    nc.gpsimd.dma_start(
        out=bpb,
        in_=bass.AP(tensor=bproj.tensor, offset=bproj.offset, ap=[[0, 128]] + list(bproj.ap)),
    )

    # ---- phase B: V GEMM (natural layout, head-strided with ones column) ----
    v2s = []
    for nt in range(NT):
        v2s.append(persist.tile([128, H * (HD + 1)], BF16, tag=f"v2{nt}", name=f"v2{nt}"))
    for nt in range(NT):
        psv = ps_big.tile([128, 1024], F32, tag="ps_big")
        for kc in range(KC):
            lhsT = xTs[kc][:, nt * 128 : (nt + 1) * 128]
            nc.tensor.matmul(
                psv[:, 0:512], lhsT, wvs[kc][:, 0:512], start=(kc == 0), stop=(kc == KC - 1)
            )
            nc.tensor.matmul(
                psv[:, 512:768], lhsT, wvs[kc][:, 512:768], start=(kc == 0), stop=(kc == KC - 1)
            )
        v2v = v2s[nt].rearrange("p (h e) -> p h e", e=HD + 1)
        nc.vector.tensor_copy(
            out=v2v[:, :, 0:HD], in_=psv[:, 0:768].rearrange("p (h e) -> p h e", e=HD)
        )
        nc.vector.memset(v2v[:, :, HD : HD + 1], 1.0)

    # ---- phases C (qk GEMM) + D (attention), interleaved per head pair ----
    def emit_qk(j, qT, kT):
        for dst, dcol0 in ((qT, j * 128), (kT, C + j * 128)):
            psq = ps_big.tile([128, 1024], F32, tag="ps_big", name="psqk")
            for kc in range(KC):
                w = wqk_pool.tile([128, 128], F32R, tag="wqk")
                nc.sync.dma_start(
                    out=w, in_=wqkvT[kc * 128 : (kc + 1) * 128, dcol0 : dcol0 + 128]
                )
                for h2 in range(NQT):
                    nc.tensor.matmul(
                        psq[:, h2 * 512 : (h2 + 1) * 512],
                        w,
                        xTs[kc][:, h2 * 512 : (h2 + 1) * 512],
                        start=(kc == 0),
                        stop=(kc == KC - 1),
                    )
            nc.vector.tensor_copy(out=dst, in_=psq)

    def emit_attn(j, qT, kT, aT):
        NG = NT // 2  # 2-chunk groups
        for nq in range(NQT):
            oA = ps_o.tile([HD + 1, 512], F32, tag="ps_o")
            oB = ps_o.tile([HD + 1, 512], F32, tag="ps_o")
            sab = [None] * NG
            eab = [None] * NG
            # software pipeline: emit S/exp for group g+0, PV for group g-1,
            # so the in-order PE never waits on ACT except at fill/drain.
            for g in range(NG + 1):
                if g < NG:
                    sA = ps_big.tile([128, 1024], F32, tag="ps_big", name="sA")
                    sB = ps_big.tile([128, 1024], F32, tag="ps_big", name="sB")
                    for half, s, kt0 in ((0, sA, 0), (1, sB, 64)):
                        for c2 in range(2):
                            nkc = 2 * g + c2
                            nc.tensor.matmul(
                                s[:, c2 * 512 : (c2 + 1) * 512],
                                kT[kt0 : kt0 + 64, nkc * 128 : (nkc + 1) * 128],
                                qT[kt0 : kt0 + 64, nq * 512 : (nq + 1) * 512],
                                tile_position=(kt0, 0),
                            )
                    eA = expp.tile([128, 1024], BF16, tag="expp", name="eA")
                    eB = expp.tile([128, 1024], BF16, tag="expp", name="eB")
                    nc.scalar.activation(out=eA, in_=sA, func=EXP, scale=SCALE)
                    nc.scalar.activation(out=eB, in_=sB, func=EXP, scale=SCALE)
                    sab[g] = (sA, sB)
                    eab[g] = (eA, eB)
                if g > 0:
                    eA, eB = eab[g - 1]
                    for c2 in range(2):
                        nkc = 2 * (g - 1) + c2
                        v2v = v2s[nkc].rearrange("p (h e) -> p h e", e=HD + 1)
                        nc.tensor.matmul(
                            oA,
                            v2v[:, 2 * j, :],
                            eA[:, c2 * 512 : (c2 + 1) * 512],
                            start=(nkc == 0),
                            stop=(nkc == NT - 1),
                        )
                        nc.tensor.matmul(
                            oB,
                            v2v[:, 2 * j + 1, :],
                            eB[:, c2 * 512 : (c2 + 1) * 512],
                            start=(nkc == 0),
                            stop=(nkc == NT - 1),
                        )
            for o, half in ((oA, 0), (oB, 1)):
                # denom row -> SBUF, bounce through DRAM to spread the 512
                # values across 128 partitions (reciprocal is ~6 cyc/elem on a
                # single DVE lane; [128,4] uses 128 lanes), bounce back and
                # broadcast-read over 64 partitions.
                rd = rdp.tile([1, 512], F32, tag="rd")
                nc.vector.tensor_copy(out=rd, in_=o[HD : HD + 1, :])
                scr = dram_scr.tile([1, 512], F32, tag="scr")
                nc.sync.dma_start(out=scr, in_=rd)
                rs = rdp.tile([128, 4], F32, tag="rs")
                nc.sync.dma_start(
                    out=rs,
                    in_=bass.AP(tensor=scr.tensor, offset=scr.offset, ap=[[4, 128], [1, 4]]),
                )
                rs2 = rdp.tile([128, 4], F32, tag="rs2")
                nc.vector.reciprocal(out=rs2, in_=rs)
                scr2 = dram_scr.tile([1, 512], F32, tag="scr2")
                nc.sync.dma_start(
                    out=bass.AP(tensor=scr2.tensor, offset=scr2.offset, ap=[[4, 128], [1, 4]]),
                    in_=rs2,
                )
                rb = rdp.tile([64, 512], F32, tag="rb")
                nc.gpsimd.dma_start(
                    out=rb,
                    in_=bass.AP(tensor=scr2.tensor, offset=scr2.offset, ap=[[0, 64]] + list(scr2.ap[1:])),
                )
                nc.vector.tensor_mul(
                    out=aT[half * 64 : half * 64 + 64, nq * 512 : (nq + 1) * 512],
                    in0=o[0:HD, :],
                    in1=rb,
                )

    aTs = []
    for j in range(PAIRS):
        qT = persist.tile([128, N], BF16, tag=f"qT{j}")
        kT = persist.tile([128, N], BF16, tag=f"kT{j}")
        aT = persist.tile([128, N], F32R, tag=f"aT{j}")
        aTs.append(aT)
        emit_qk(j, qT, kT)
        emit_attn(j, qT, kT, aT)

    # ---- phase E: proj + bias ----
    for nt in range(NT):
        psy = ps_big.tile([128, 1024], F32, tag="ps_big", name="psy")
        for kc in range(KC):
            lhsT = aTs[kc][:, nt * 128 : (nt + 1) * 128]
            nc.tensor.matmul(
                psy[:, 0:512], lhsT, wps[kc][:, 0:512], start=(kc == 0), stop=(kc == KC - 1)
            )
            nc.tensor.matmul(
                psy[:, 512:768], lhsT, wps[kc][:, 512:768], start=(kc == 0), stop=(kc == KC - 1)
            )
        yb = work.tile([128, C], F32, tag="yb")
        nc.vector.tensor_add(out=yb, in0=psy[:, 0:768], in1=bpb)
        nc.sync.dma_start(out=y[nt * 128 : (nt + 1) * 128, :], in_=yb)


def build():
    from contextlib import ExitStack

    nc = bacc.Bacc("TRN2", target_bir_lowering=False, debug=False)
    xT = nc.dram_tensor("xT", [C, N], F32R, kind="ExternalInput").ap()
    wqkvT = nc.dram_tensor("wqkvT", [C, 3 * C], F32R, kind="ExternalInput").ap()
    wprojT = nc.dram_tensor("wprojT", [C, C], F32R, kind="ExternalInput").ap()
    bproj = nc.dram_tensor("bproj", [C], F32, kind="ExternalInput").ap()
    y = nc.dram_tensor("y", [N, C], F32, kind="ExternalOutput").ap()
    with tile.TileContext(nc) as tc:
        with ExitStack() as ctx:
            _emit(tc, nc, xT, wqkvT, wprojT, bproj, y, ctx)
    nc.compile()
    return nc


_NC_CACHE = {}


def make_in_maps(x, w_qkv, w_proj, b_proj):
    wqkvT = np.ascontiguousarray(np.asarray(w_qkv).T)
    wprojT = np.ascontiguousarray(np.asarray(w_proj).T)
    b_proj = np.asarray(b_proj)
    return [
        {
            "xT": np.ascontiguousarray(np.asarray(x[b]).T),
            "wqkvT": wqkvT,
            "wprojT": wprojT,
            "bproj": b_proj,
        }
        for b in range(NCORES)
    ]


def kernel(x, w_qkv, w_proj, b_proj, _trace=False, _tmpdir=None):
    if "nc" not in _NC_CACHE:
        _NC_CACHE["nc"] = build()
    nc = _NC_CACHE["nc"]
    in_maps = make_in_maps(x, w_qkv, w_proj, b_proj)
    kwargs = {}
    if _trace:
        kwargs = {"trace": True, "tmpdir": _tmpdir}
    res = run_bass_kernel_spmd(nc, in_maps, core_ids=list(range(NCORES)), **kwargs)
    out = np.stack([res.results[i]["y"] for i in range(NCORES)], axis=0)
    if _trace:
        _NC_CACHE["last_result"] = res
    return out


if __name__ == "__main__":
    rng = np.random.default_rng(0)
    x = rng.standard_normal((B, N, C), dtype=np.float32)
    w_qkv = (rng.standard_normal((3 * C, C), dtype=np.float32) * C**-0.5).astype(np.float32)
    w_proj = (rng.standard_normal((C, C), dtype=np.float32) * C**-0.5).astype(np.float32)
    b_proj = np.zeros(C, dtype=np.float32)
    out = kernel(x, w_qkv, w_proj, b_proj)
    print("out", out.shape, out.dtype, float(np.abs(out).mean()))


# revision 11
# speedup vs baseline: 1.8069x; 1.3655x over previous
"""Multi-head attention (B=8, N=1024, C=768, H=12) for 8 Trainium2 NeuronCores.

Sharding: data-parallel over the batch dim — core b computes batch element b.
Weights are replicated; no collectives.

Per-core plan (all layouts picked so that NO on-device transposes are needed):
  host feeds xT=[C,N] (x[b].T), wqkvT=[C,3C], wprojT=[C,C], bproj=[C].
  1. qT/kT GEMM:  qkT[d, n] = wqkvT_chunk.T @ xT          (d on partitions)
  2. V GEMM:      V[n, dv]  = xT_chunk.T @ wqkvT[:, 2C:]  (natural layout),
                  stored head-strided as V''[n, 12*(64+1)] with a ones column
                  per head (the ones row of V''.T yields the softmax
                  denominator for free during the PV matmul).
  3. Attention per head pair (2 heads packed into the 128-wide PE array by
     row tiling, since head_dim=64):
       S^T[nk, nq]  = kT_chunk.T @ qT          (one matmul per nk chunk)
       expS         = exp(SCALE * S^T)         (ACT engine, PSUM -> SBUF;
                                               max-free softmax: scaled
                                               scores are ~N(0,1), fp32-safe)
       O'[65, nq]  += V''_chunk.T @ expS       (rows 0:64 = unnormalized out^T,
                                               row 64 = sum of exp = denom)
       attnT[c, n]  = O'[0:64] * (1/denom)     (reciprocal + partition-bcast)
  4. proj:        y[n, d2] = attnT_chunk.T @ wprojT + bproj
All matmuls run as float32r (full PE rate at N>=256, ~fp32 precision).
"""

import sys

for _p in ("/opt/trn_rl_repo", "/opt/pypackages"):
    if _p not in sys.path:
        sys.path.append(_p)

import numpy as np

import concourse.bass as bass
import concourse.tile as tile
from concourse import bacc, mybir
from concourse.bass_utils import run_bass_kernel_spmd

B, N, C = 8, 1024, 768
H, HD = 12, 64
SCALE = HD**-0.5
NCORES = 8
KC = C // 128  # 6 contraction chunks over C
NT = N // 128  # 8 chunks over sequence (nk / n-tiles)
NQT = N // 512  # 2 moving-dim tiles over the query sequence
PAIRS = H // 2  # 6 head pairs
F32 = mybir.dt.float32
F32R = mybir.dt.float32r
BF16 = mybir.dt.bfloat16
EXP = mybir.ActivationFunctionType.Exp


def _emit(tc, nc, xT, wqkvT, wprojT, bproj, y, ctx):
    persist = ctx.enter_context(tc.tile_pool(name="persist", bufs=1))
    wqk_pool = ctx.enter_context(tc.tile_pool(name="wqk", bufs=12))
    work = ctx.enter_context(tc.tile_pool(name="work", bufs=3))
    expp = ctx.enter_context(tc.tile_pool(name="expp", bufs=4))
    rdp = ctx.enter_context(tc.tile_pool(name="rdp", bufs=4))
    dram_scr = ctx.enter_context(tc.tile_pool(name="dram_scr", bufs=4, space="DRAM"))
    # 8 PSUM banks total: ps_big = 3 slots x [128,1024] (2 banks each),
    # ps_o = 2 slots x [65,512] (1 bank each).
    ps_big = ctx.enter_context(tc.tile_pool(name="ps_big", bufs=3, space="PSUM"))
    ps_o = ctx.enter_context(tc.tile_pool(name="ps_o", bufs=2, space="PSUM"))

    # ---- persistent loads ----
    xTs = []
    for kc in range(KC):
        t = persist.tile([128, N], F32R, tag=f"xT{kc}")
        nc.sync.dma_start(out=t, in_=xT[kc * 128 : (kc + 1) * 128, :])
        xTs.append(t)
    wvs = []
    for kc in range(KC):
        t = persist.tile([128, C], F32R, tag=f"wv{kc}")
        nc.sync.dma_start(out=t, in_=wqkvT[kc * 128 : (kc + 1) * 128, 2 * C : 3 * C])
        wvs.append(t)
    wps = []
    for kc in range(KC):
        t = persist.tile([128, C], F32R, tag=f"wp{kc}")
        nc.sync.dma_start(out=t, in_=wprojT[kc * 128 : (kc + 1) * 128, :])
        wps.append(t)
    bpb = persist.tile([128, C], F32, tag="bpb")
    nc.gpsimd.dma_start(
        out=bpb,
        in_=bass.AP(tensor=bproj.tensor, offset=bproj.offset, ap=[[0, 128]] + list(bproj.ap)),
    )

    # ---- phase B: V GEMM (natural layout, head-strided with ones column) ----
    v2s = []
    for nt in range(NT):
        v2s.append(persist.tile([128, H * (HD + 1)], BF16, tag=f"v2{nt}", name=f"v2{nt}"))
    for nt in range(NT):
        psv = ps_big.tile([128, 1024], F32, tag="ps_big")
        for kc in range(KC):
            lhsT = xTs[kc][:, nt * 128 : (nt + 1) * 128]
            nc.tensor.matmul(
                psv[:, 0:512], lhsT, wvs[kc][:, 0:512], start=(kc == 0), stop=(kc == KC - 1)
            )
            nc.tensor.matmul(
                psv[:, 512:768], lhsT, wvs[kc][:, 512:768], start=(kc == 0), stop=(kc == KC - 1)
            )
        v2v = v2s[nt].rearrange("p (h e) -> p h e", e=HD + 1)
        nc.vector.tensor_copy(
            out=v2v[:, :, 0:HD], in_=psv[:, 0:768].rearrange("p (h e) -> p h e", e=HD)
        )
        nc.vector.memset(v2v[:, :, HD : HD + 1], 1.0)

    # ---- phases C (qk GEMM) + D (attention), interleaved per head pair ----
    def emit_qk(j, qT, kT):
        for dst, dcol0 in ((qT, j * 128), (kT, C + j * 128)):
            psq = ps_big.tile([128, 1024], F32, tag="ps_big", name="psqk")
            for kc in range(KC):
                w = wqk_pool.tile([128, 128], F32R, tag="wqk")
                nc.sync.dma_start(
                    out=w, in_=wqkvT[kc * 128 : (kc + 1) * 128, dcol0 : dcol0 + 128]
                )
                for h2 in range(NQT):
                    nc.tensor.matmul(
                        psq[:, h2 * 512 : (h2 + 1) * 512],
                        w,
                        xTs[kc][:, h2 * 512 : (h2 + 1) * 512],
                        start=(kc == 0),
                        stop=(kc == KC - 1),
                    )
            nc.vector.tensor_copy(out=dst, in_=psq)

    def emit_attn(j, qT, kT, aT):
        NG = NT // 2  # 2-chunk groups
        for nq in range(NQT):
            oA = ps_o.tile([HD + 1, 512], F32, tag="ps_o")
            oB = ps_o.tile([HD + 1, 512], F32, tag="ps_o")
            sab = [None] * NG
            eab = [None] * NG
            # software pipeline: emit S/exp for group g+0, PV for group g-1,
            # so the in-order PE never waits on ACT except at fill/drain.
            for g in range(NG + 1):
                if g < NG:
                    sA = ps_big.tile([128, 1024], F32, tag="ps_big", name="sA")
                    sB = ps_big.tile([128, 1024], F32, tag="ps_big", name="sB")
                    for half, s, kt0 in ((0, sA, 0), (1, sB, 64)):
                        for c2 in range(2):
                            nkc = 2 * g + c2
                            nc.tensor.matmul(
                                s[:, c2 * 512 : (c2 + 1) * 512],
                                kT[kt0 : kt0 + 64, nkc * 128 : (nkc + 1) * 128],
                                qT[kt0 : kt0 + 64, nq * 512 : (nq + 1) * 512],
                                tile_position=(kt0, 0),
                            )
                    eA = expp.tile([128, 1024], BF16, tag="expp", name="eA")
                    eB = expp.tile([128, 1024], BF16, tag="expp", name="eB")
                    nc.scalar.activation(out=eA, in_=sA, func=EXP, scale=SCALE)
                    nc.scalar.activation(out=eB, in_=sB, func=EXP, scale=SCALE)
                    sab[g] = (sA, sB)
                    eab[g] = (eA, eB)
                if g > 0:
                    eA, eB = eab[g - 1]
                    for c2 in range(2):
                        nkc = 2 * (g - 1) + c2
                        v2v = v2s[nkc].rearrange("p (h e) -> p h e", e=HD + 1)
                        nc.tensor.matmul(
                            oA,
                            v2v[:, 2 * j, :],
                            eA[:, c2 * 512 : (c2 + 1) * 512],
                            start=(nkc == 0),
                            stop=(nkc == NT - 1),
                        )
                        nc.tensor.matmul(
                            oB,
                            v2v[:, 2 * j + 1, :],
                            eB[:, c2 * 512 : (c2 + 1) * 512],
                            start=(nkc == 0),
                            stop=(nkc == NT - 1),
                        )
            for o, half in ((oA, 0), (oB, 1)):
                # Drain O' to SBUF immediately so the PSUM bank frees for the
                # next nq tile (the reciprocal chain below is ~4us deep).
                oc = rdp.tile([HD + 1, 512], F32, tag="oc")
                nc.vector.tensor_copy(out=oc, in_=o)
                # denom row: bounce through DRAM to spread the 512 values
                # across 128 partitions (reciprocal is ~6 cyc/elem on a
                # single DVE lane; [128,4] uses 128 lanes), bounce back and
                # broadcast-read over 64 partitions.
                scr = dram_scr.tile([1, 512], F32, tag="scr")
                nc.sync.dma_start(out=scr, in_=oc[HD : HD + 1, :])
                rs = rdp.tile([128, 4], F32, tag="rs")
                nc.sync.dma_start(
                    out=rs,
                    in_=bass.AP(tensor=scr.tensor, offset=scr.offset, ap=[[4, 128], [1, 4]]),
                )
                rs2 = rdp.tile([128, 4], F32, tag="rs2")
                nc.vector.reciprocal(out=rs2, in_=rs)
                scr2 = dram_scr.tile([1, 512], F32, tag="scr2")
                nc.sync.dma_start(
                    out=bass.AP(tensor=scr2.tensor, offset=scr2.offset, ap=[[4, 128], [1, 4]]),
                    in_=rs2,
                )
                rb = rdp.tile([64, 512], F32, tag="rb")
                nc.gpsimd.dma_start(
                    out=rb,
                    in_=bass.AP(tensor=scr2.tensor, offset=scr2.offset, ap=[[0, 64]] + list(scr2.ap[1:])),
                )
                nc.vector.tensor_mul(
                    out=aT[half * 64 : half * 64 + 64, nq * 512 : (nq + 1) * 512],
                    in0=oc[0:HD, :],
                    in1=rb,
                )

    aTs = []
    for j in range(PAIRS):
        qT = persist.tile([128, N], BF16, tag=f"qT{j}")
        kT = persist.tile([128, N], BF16, tag=f"kT{j}")
        aT = persist.tile([128, N], F32R, tag=f"aT{j}")
        aTs.append(aT)
        emit_qk(j, qT, kT)
        emit_attn(j, qT, kT, aT)

    # ---- phase E: proj + bias ----
    for nt in range(NT):
        psy = ps_big.tile([128, 1024], F32, tag="ps_big", name="psy")
        for kc in range(KC):
            lhsT = aTs[kc][:, nt * 128 : (nt + 1) * 128]
            nc.tensor.matmul(
                psy[:, 0:512], lhsT, wps[kc][:, 0:512], start=(kc == 0), stop=(kc == KC - 1)
            )
            nc.tensor.matmul(
                psy[:, 512:768], lhsT, wps[kc][:, 512:768], start=(kc == 0), stop=(kc == KC - 1)
            )
        yb = work.tile([128, C], F32, tag="yb")
        nc.vector.tensor_add(out=yb, in0=psy[:, 0:768], in1=bpb)
        nc.sync.dma_start(out=y[nt * 128 : (nt + 1) * 128, :], in_=yb)


def build():
    from contextlib import ExitStack

    nc = bacc.Bacc("TRN2", target_bir_lowering=False, debug=False)
    xT = nc.dram_tensor("xT", [C, N], F32R, kind="ExternalInput").ap()
    wqkvT = nc.dram_tensor("wqkvT", [C, 3 * C], F32R, kind="ExternalInput").ap()
    wprojT = nc.dram_tensor("wprojT", [C, C], F32R, kind="ExternalInput").ap()
    bproj = nc.dram_tensor("bproj", [C], F32, kind="ExternalInput").ap()
    y = nc.dram_tensor("y", [N, C], F32, kind="ExternalOutput").ap()
    with tile.TileContext(nc) as tc:
        with ExitStack() as ctx:
            _emit(tc, nc, xT, wqkvT, wprojT, bproj, y, ctx)
    nc.compile()
    return nc


_NC_CACHE = {}


def make_in_maps(x, w_qkv, w_proj, b_proj):
    wqkvT = np.ascontiguousarray(np.asarray(w_qkv).T)
    wprojT = np.ascontiguousarray(np.asarray(w_proj).T)
    b_proj = np.asarray(b_proj)
    return [
        {
            "xT": np.ascontiguousarray(np.asarray(x[b]).T),
            "wqkvT": wqkvT,
            "wprojT": wprojT,
            "bproj": b_proj,
        }
        for b in range(NCORES)
    ]


def kernel(x, w_qkv, w_proj, b_proj, _trace=False, _tmpdir=None):
    if "nc" not in _NC_CACHE:
        _NC_CACHE["nc"] = build()
    nc = _NC_CACHE["nc"]
    in_maps = make_in_maps(x, w_qkv, w_proj, b_proj)
    kwargs = {}
    if _trace:
        kwargs = {"trace": True, "tmpdir": _tmpdir}
    res = run_bass_kernel_spmd(nc, in_maps, core_ids=list(range(NCORES)), **kwargs)
    out = np.stack([res.results[i]["y"] for i in range(NCORES)], axis=0)
    if _trace:
        _NC_CACHE["last_result"] = res
    return out


if __name__ == "__main__":
    rng = np.random.default_rng(0)
    x = rng.standard_normal((B, N, C), dtype=np.float32)
    w_qkv = (rng.standard_normal((3 * C, C), dtype=np.float32) * C**-0.5).astype(np.float32)
    w_proj = (rng.standard_normal((C, C), dtype=np.float32) * C**-0.5).astype(np.float32)
    b_proj = np.zeros(C, dtype=np.float32)
    out = kernel(x, w_qkv, w_proj, b_proj)
    print("out", out.shape, out.dtype, float(np.abs(out).mean()))
